# revision 18
# baseline (speedup 1.0000x reference)
"""MoE kernel for TRN2, 8 NeuronCores, data-parallel over the batch dim.

Reference computation (B=8192, D=1024, H=1024, E=16):
    weights = softmax(x @ Wg + bg, axis=1)            # [B, E]
    h       = relu(einsum('bd,edh->beh', x, W1) + b1) # [B, E, H]
    eo      = einsum('beh,eh->be', h, W2) + b2        # [B, E]
    out     = sum(eo * weights, axis=1, keepdims=True)# [B, 1]

Strategy (v2):
  - Shard B over 8 cores (1024 rows/core); weights replicated.
  - Everything in bf16 on the PE (tolerance 2e-2 vs ~3e-3 bf16 noise):
    halves W1 DMA traffic and enables fast weight load (FWL).
  - Stage 1 per t=(ht,e): psum[h=128, b=512x2] accumulated over 8 d-tiles.
  - ReLU+b1 on ScalarE -> hr bf16.
  - Stage 2 uses PE column tiling: experts are assigned to the 4 32-col
    groups (e -> group e%4, row e//4), so 4 consecutive t's stage-2
    matmuls run CONCURRENTLY in disjoint col groups (~4x faster than a
    serialized block-diagonal form). Emitted in bursts of 4 t's, one t
    delayed so ReLU has time to land.
  - Gating: stationary Wg produces logits directly as [16e, B] (expert
    e at partition 32*(e%4)+e//4); U = exp(logits + bg) UNNORMALIZED.
    den = sum_e U and num2 = sum_e U*b2 via tiny partition-sum matmuls.
  - Combine: V = U * eo (DVE) -> num1 via 8 partition-sum matmuls ->
    y = (num1 + num2) / den -> single [128, 8] DMA out.
"""

import ml_dtypes
import numpy as np

import concourse.bacc as bacc
import concourse.bass as bass
import concourse.mybir as mybir
from concourse import tile
from concourse.bass_utils import run_bass_kernel_spmd

B, D, H, E = 8192, 1024, 1024, 16
N_CORES = 8
BS = B // N_CORES  # 1024 batch rows per core
BH = 512           # half-batch moving-operand width (one psum bank)
DT = D // 128      # 8 d-tiles
HT = H // 128      # 8 h-tiles
T = E * HT         # 128 tiles; t = ht*16 + e  (e minor)
NCH = BS // 128    # 8 b-chunks of 128

F32 = mybir.dt.float32
BF16 = mybir.dt.bfloat16
AF = mybir.ActivationFunctionType
BF_NP = ml_dtypes.bfloat16


def build_bass():
    nc = bacc.Bacc("TRN2", target_bir_lowering=False, debug=False)
    xt_d = nc.dram_tensor("xt", [128, DT * BS], BF16, kind="ExternalInput")
    w1_d = nc.dram_tensor("w1p", [T, 128, DT * 128], BF16, kind="ExternalInput")
    b1t_d = nc.dram_tensor("b1t", [128, T], F32, kind="ExternalInput")
    w2c_d = nc.dram_tensor("w2c", [128, T * 32], BF16, kind="ExternalInput")
    wgp_d = nc.dram_tensor("wgp4", [128, DT * 128], BF16, kind="ExternalInput")
    bg4_d = nc.dram_tensor("bg4", [128, 1], F32, kind="ExternalInput")
    ob2_d = nc.dram_tensor("ob2", [128, 2], BF16, kind="ExternalInput")
    y_d = nc.dram_tensor("y", [128, NCH], F32, kind="ExternalOutput")

    with tile.TileContext(nc) as tc:
        with (
            tc.tile_pool(name="const", bufs=1) as cpool,
            tc.tile_pool(name="w1", bufs=8) as w1pool,
            tc.tile_pool(name="hrelu", bufs=12) as hpool,
            tc.tile_pool(name="sm", bufs=2) as smpool,
            tc.tile_pool(name="ps_h", bufs=2, space=bass.MemorySpace.PSUM) as psh,
            tc.tile_pool(name="ps_eo", bufs=1, space=bass.MemorySpace.PSUM) as pseo,
            tc.tile_pool(name="ps_s", bufs=1, space=bass.MemorySpace.PSUM) as pss,
        ):
            # ---- resident tensors; xt is split across BOTH HWDGE rings
            # (scalar=ACT gets d0..3, sync=SP gets d4..7 ahead of the W1
            # stream) so gating can start ~2x sooner ----
            # ring order: only wgp ahead of xt; all other consts after
            # (each dma_start costs ~0.7us of ring issue time)
            wgp_sb = cpool.tile([128, DT * 128], BF16, tag="wgp")
            nc.scalar.dma_start(wgp_sb[:], wgp_d[:])
            w1_first = w1pool.tile([128, DT * 128], BF16, tag="w1t")
            nc.sync.dma_start(w1_first[:], w1_d[0, :, :])
            xt_sb = cpool.tile([128, DT * BS], BF16, tag="xt")
            for dd in range(DT):
                eng = nc.scalar if dd < 4 else nc.sync
                eng.dma_start(
                    xt_sb[:, dd * BS:(dd + 1) * BS], xt_d[:, dd * BS:(dd + 1) * BS]
                )
            b1t_sb = cpool.tile([128, T], F32, tag="b1t")
            nc.scalar.dma_start(b1t_sb[:], b1t_d[:])
            bg4_sb = cpool.tile([128, 1], F32, tag="bg4")
            nc.scalar.dma_start(bg4_sb[:], bg4_d[:])
            ob2_sb = cpool.tile([128, 2], BF16, tag="ob2")
            nc.scalar.dma_start(ob2_sb[:], ob2_d[:])
            w2c_sb = cpool.tile([128, T * 32], BF16, tag="w2c")
            nc.scalar.dma_start(w2c_sb[:], w2c_d[:])

            # gating is interleaved with stage-1 t=0 inside the main loop.
            # gps lives in the eo pool (bufs=1, so it aliases eo_ps): the
            # Exp read finishes by ~21us and eo's first burst (t=8, all
            # start=True) fully reinitializes the banks — and this keeps
            # ps1's two psh buffers free of WAR stalls against Exp.
            gps = pseo.tile([128, BS], F32, tag="eo")
            gate_order = [0, 4, 1, 5, 2, 6, 3, 7]
            u4 = cpool.tile([128, BS], BF16, tag="u4")
            u4f = cpool.tile([128, BS], F32, tag="u4f")

            # HAM warm-up: dependency-free matmuls on an uninitialized
            # scratch tile run during the initial DMA wait, so the PE is
            # at full clock (K=8/8) when real work arrives. Results are
            # overwritten by the gating group's start=True.
            scratch = cpool.tile([128, BH], BF16, tag="scratch")
            nc.gpsimd.memset(scratch[:], 0.0)
            for _ in range(16):
                nc.tensor.matmul(
                    gps[:, 0:BH],
                    scratch[:, 0:128],
                    scratch[:],
                    start=True, stop=True, skip_group_check=True,
                )

            # ---- main loop over t = ht*16 + e ----
            eo_ps = pseo.tile([128, BS], F32, tag="eo")
            pending = []  # [(t, hr), ...] up to BURST
            BURST = 8

            def emit_stage2_burst():
                # consecutive t's alternate col groups, so MMs overlap
                for bh in range(2):
                    for (tt, hh) in pending:
                        g = (tt % E) % 4
                        nc.tensor.matmul(
                            eo_ps[32 * g:32 * g + 32, bh * BH:(bh + 1) * BH],
                            w2c_sb[:, tt * 32:(tt + 1) * 32],
                            hh[:, bh * BH:(bh + 1) * BH],
                            start=(tt < 4), stop=(tt >= T - 4),
                            skip_group_check=True,
                            tile_position=(0, 32 * g),
                        )
                pending.clear()

            den_ps = pss.tile([128, NCH], F32, tag="sps")
            nb2_ps = pss.tile([128, NCH], F32, tag="sps2")
            rden = cpool.tile([128, NCH], F32, tag="rden")
            num2 = cpool.tile([128, NCH], F32, tag="num2")

            for t in range(T):
                if t == 0:
                    w1t = w1_first
                else:
                    w1t = w1pool.tile([128, DT * 128], BF16, tag="w1t")
                    nc.sync.dma_start(w1t[:], w1_d[t, :, :])
                ps1 = psh.tile([128, BS], F32, tag="ps1")
                # t=0: follow the two xt DMA streams, interleaving the
                # gating matmuls so the PE has work as tiles arrive
                dds = gate_order if t == 0 else range(DT)
                for i, dd in enumerate(dds):
                    if t == 0:
                        for bh in range(2):
                            nc.tensor.matmul(
                                gps[:, bh * BH:(bh + 1) * BH],
                                wgp_sb[:, dd * 128:(dd + 1) * 128],
                                xt_sb[:, dd * BS + bh * BH: dd * BS + (bh + 1) * BH],
                                start=(i == 0), stop=(i == DT - 1),
                                skip_group_check=True,
                            )
                    lhs = w1t[:, dd * 128:(dd + 1) * 128]
                    for bh in range(2):
                        nc.tensor.matmul(
                            ps1[:, bh * BH:(bh + 1) * BH],
                            lhs,
                            xt_sb[:, dd * BS + bh * BH: dd * BS + (bh + 1) * BH],
                            start=(i == 0), stop=(i == DT - 1),
                            skip_group_check=True,
                        )
                if t == 0:
                    nc.scalar.activation(u4[:], gps[:], AF.Exp, bias=bg4_sb[:])
                    nc.vector.tensor_copy(u4f[:], u4[:])
                if len(pending) == BURST or t >= T - 2:
                    # flush early near the end so the final bursts never
                    # wait on a ReLU still in the ACT queue
                    emit_stage2_burst()
                if t == 1:
                    # den/num2 partition-sums; u4 is ready by now, PE is warm
                    for j in range(NCH):
                        nc.tensor.matmul(
                            den_ps[:, j:j + 1],
                            u4[:, j * 128:(j + 1) * 128],
                            ob2_sb[:, 0:1],
                            start=True, stop=True, skip_group_check=True,
                        )
                        nc.tensor.matmul(
                            nb2_ps[:, j:j + 1],
                            u4[:, j * 128:(j + 1) * 128],
                            ob2_sb[:, 1:2],
                            start=True, stop=True, skip_group_check=True,
                        )
                if t == 2:
                    nc.vector.reciprocal(rden[:], den_ps[:])
                    nc.vector.tensor_copy(num2[:], nb2_ps[:])
                hr = hpool.tile([128, BS], BF16, tag="hr")
                # final tile: quarter-width ReLU chunks so the last stage-2
                # matmuls can chase them with minimal PE wait
                nq = 4 if t == T - 1 else 2
                w = BS // nq
                for q in range(nq):
                    nc.scalar.activation(
                        hr[:, q * w:(q + 1) * w],
                        ps1[:, q * w:(q + 1) * w],
                        AF.Relu,
                        bias=b1t_sb[:, t:t + 1],
                    )
                pending.append((t, hr))
            (tt, hh) = pending.pop()
            g = (tt % E) % 4
            for q in range(4):
                nc.tensor.matmul(
                    eo_ps[32 * g:32 * g + 32, q * 256:(q + 1) * 256],
                    w2c_sb[:, tt * 32:(tt + 1) * 32],
                    hh[:, q * 256:(q + 1) * 256],
                    start=False, stop=True,
                    skip_group_check=True,
                    tile_position=(0, 32 * g),
                )

            # ---- combine: V = U*eo -> num1 -> y = (num1+num2)*rden ----
            # V-mul split in halves so the partition-sum matmuls for the
            # first half overlap the DVE on the second half
            v4 = cpool.tile([128, BS], BF16, tag="v4")
            num_ps = pss.tile([128, NCH], F32, tag="sps")
            for qq in range(4):
                cols = slice(qq * 256, (qq + 1) * 256)
                nc.vector.tensor_mul(v4[:, cols], eo_ps[:, cols], u4f[:, cols])
                for j in (2 * qq, 2 * qq + 1):
                    nc.tensor.matmul(
                        num_ps[:, j:j + 1],
                        v4[:, j * 128:(j + 1) * 128],
                        ob2_sb[:, 0:1],
                        start=True, stop=True, skip_group_check=True,
                    )
            ysb = smpool.tile([128, NCH], F32, tag="ysb")
            nc.vector.tensor_add(ysb[:], num_ps[:], num2[:])
            nc.vector.tensor_mul(ysb[:], ysb[:], rden[:])
            nc.sync.dma_start(y_d[:], ysb[:])
    nc.compile()
    return nc


def prep_inputs(x, W1, b1, W2, b2, Wg, bg):
    """Host-side data prep. Returns (shared_map, per_core_xt)."""
    f = np.float32
    # W1 [E, D, H] -> [t=(ht,e), d_in, (d_t, h_in)]
    w1p = np.ascontiguousarray(
        W1.reshape(E, DT, 128, HT, 128).transpose(3, 0, 2, 1, 4)
        .reshape(T, 128, DT * 128)).astype(BF_NP)
    b1t = np.ascontiguousarray(
        b1.reshape(E, HT, 128).transpose(2, 1, 0).reshape(128, T).astype(f))
    # stage-2 stationaries: expert e -> col group g=e%4, row k=e//4
    w2c = np.zeros((128, T, 32), dtype=f)
    for t in range(T):
        ht, e = divmod(t, E)
        k, g = divmod(e, 4)
        w2c[:, t, k] = W2[e, ht * 128:(ht + 1) * 128]
    w2c = w2c.reshape(128, T * 32).astype(BF_NP)
    # gating stationary: col 32g+k = Wg[:, 4k+g], rest zero
    wgp4 = np.zeros((DT, 128, 128), dtype=f)
    bg4 = np.full((128, 1), -30.0, dtype=f)
    ob2 = np.zeros((128, 2), dtype=f)
    ob2[:, 0] = 1.0
    for e in range(E):
        k, g = divmod(e, 4)
        wgp4[:, :, 32 * g + k] = Wg[:, e].reshape(DT, 128)
        bg4[32 * g + k, 0] = bg[e]
        ob2[32 * g + k, 1] = b2[e]
    wgp4 = np.ascontiguousarray(
        wgp4.transpose(1, 0, 2).reshape(128, DT * 128)).astype(BF_NP)
    ob2 = ob2.astype(BF_NP)
    shared = {"w1p": w1p, "b1t": b1t, "w2c": w2c, "wgp4": wgp4,
              "bg4": bg4, "ob2": ob2}
    xT = np.ascontiguousarray(np.asarray(x, dtype=f).T)  # [D, B]
    xts = []
    for c in range(N_CORES):
        xc = xT[:, c * BS:(c + 1) * BS]  # [D, BS]
        xc = np.ascontiguousarray(
            xc.reshape(DT, 128, BS).transpose(1, 0, 2).reshape(128, DT * BS))
        xts.append(xc.astype(BF_NP))
    return shared, xts


def run(inputs, trace=False):
    nc = build_bass()
    shared, xts = prep_inputs(**inputs)
    in_maps = [dict(shared, xt=xts[c]) for c in range(N_CORES)]
    res = run_bass_kernel_spmd(
        nc, in_maps, core_ids=list(range(N_CORES)), trace=trace
    )
    # y dram is [128, NCH] with y[p, j] = out[j*128 + p]
    y = np.concatenate(
        [np.asarray(r["y"], dtype=np.float32).T.reshape(BS, 1)
         for r in res.results], axis=0)
    return y, res


def kernel(**inputs):
    y, _ = run(inputs, trace=False)
    return y


if __name__ == "__main__":
    rng = np.random.default_rng(0)
    ins = {
        "x": rng.standard_normal((B, D), dtype=np.float32),
        "W1": rng.standard_normal((E, D, H), dtype=np.float32) / 32,
        "b1": rng.standard_normal((E, H), dtype=np.float32) / 32,
        "W2": rng.standard_normal((E, H), dtype=np.float32) / 32,
        "b2": rng.standard_normal((E,), dtype=np.float32) / 32,
        "Wg": rng.standard_normal((D, E), dtype=np.float32) / 32,
        "bg": rng.standard_normal((E,), dtype=np.float32) / 32,
    }
    y = kernel(**ins)
    print("ok", y.shape, y.dtype)


# revision 19
# speedup vs baseline: 1.0010x; 1.0010x over previous
"""MoE kernel for TRN2, 8 NeuronCores, data-parallel over the batch dim.

Reference computation (B=8192, D=1024, H=1024, E=16):
    weights = softmax(x @ Wg + bg, axis=1)            # [B, E]
    h       = relu(einsum('bd,edh->beh', x, W1) + b1) # [B, E, H]
    eo      = einsum('beh,eh->be', h, W2) + b2        # [B, E]
    out     = sum(eo * weights, axis=1, keepdims=True)# [B, 1]

Strategy (v2):
  - Shard B over 8 cores (1024 rows/core); weights replicated.
  - Everything in bf16 on the PE (tolerance 2e-2 vs ~3e-3 bf16 noise):
    halves W1 DMA traffic and enables fast weight load (FWL).
  - Stage 1 per t=(ht,e): psum[h=128, b=512x2] accumulated over 8 d-tiles.
  - ReLU+b1 on ScalarE -> hr bf16.
  - Stage 2 uses PE column tiling: experts are assigned to the 4 32-col
    groups (e -> group e%4, row e//4), so 4 consecutive t's stage-2
    matmuls run CONCURRENTLY in disjoint col groups (~4x faster than a
    serialized block-diagonal form). Emitted in bursts of 4 t's, one t
    delayed so ReLU has time to land.
  - Gating: stationary Wg produces logits directly as [16e, B] (expert
    e at partition 32*(e%4)+e//4); U = exp(logits + bg) UNNORMALIZED.
    den = sum_e U and num2 = sum_e U*b2 via tiny partition-sum matmuls.
  - Combine: V = U * eo (DVE) -> num1 via 8 partition-sum matmuls ->
    y = (num1 + num2) / den -> single [128, 8] DMA out.
"""

import ml_dtypes
import numpy as np

import concourse.bacc as bacc
import concourse.bass as bass
import concourse.mybir as mybir
from concourse import tile
from concourse.bass_utils import run_bass_kernel_spmd

B, D, H, E = 8192, 1024, 1024, 16
N_CORES = 8
BS = B // N_CORES  # 1024 batch rows per core
BH = 512           # half-batch moving-operand width (one psum bank)
DT = D // 128      # 8 d-tiles
HT = H // 128      # 8 h-tiles
T = E * HT         # 128 tiles; t = ht*16 + e  (e minor)
NCH = BS // 128    # 8 b-chunks of 128

F32 = mybir.dt.float32
BF16 = mybir.dt.bfloat16
AF = mybir.ActivationFunctionType
BF_NP = ml_dtypes.bfloat16


def build_bass():
    nc = bacc.Bacc("TRN2", target_bir_lowering=False, debug=False)
    xt_d = nc.dram_tensor("xt", [128, DT * BS], BF16, kind="ExternalInput")
    w1_d = nc.dram_tensor("w1p", [T, 128, DT * 128], BF16, kind="ExternalInput")
    b1t_d = nc.dram_tensor("b1t", [128, T], F32, kind="ExternalInput")
    w2c_d = nc.dram_tensor("w2c", [128, T * 32], BF16, kind="ExternalInput")
    wgp_d = nc.dram_tensor("wgp4", [128, DT * 128], BF16, kind="ExternalInput")
    bg4_d = nc.dram_tensor("bg4", [128, 1], F32, kind="ExternalInput")
    ob2_d = nc.dram_tensor("ob2", [128, 2], BF16, kind="ExternalInput")
    y_d = nc.dram_tensor("y", [128, NCH], F32, kind="ExternalOutput")

    with tile.TileContext(nc) as tc:
        with (
            tc.tile_pool(name="const", bufs=1) as cpool,
            tc.tile_pool(name="w1", bufs=8) as w1pool,
            tc.tile_pool(name="hrelu", bufs=12) as hpool,
            tc.tile_pool(name="sm", bufs=2) as smpool,
            tc.tile_pool(name="ps_h", bufs=2, space=bass.MemorySpace.PSUM) as psh,
            tc.tile_pool(name="ps_eo", bufs=1, space=bass.MemorySpace.PSUM) as pseo,
            tc.tile_pool(name="ps_s", bufs=1, space=bass.MemorySpace.PSUM) as pss,
        ):
            # ---- resident tensors; xt is split across BOTH HWDGE rings
            # (scalar=ACT gets d0..3, sync=SP gets d4..7 ahead of the W1
            # stream) so gating can start ~2x sooner ----
            # ring order: only wgp ahead of xt; all other consts after
            # (each dma_start costs ~0.7us of ring issue time)
            wgp_sb = cpool.tile([128, DT * 128], BF16, tag="wgp")
            nc.scalar.dma_start(wgp_sb[:], wgp_d[:])
            w1_first = w1pool.tile([128, DT * 128], BF16, tag="w1t")
            nc.sync.dma_start(w1_first[:], w1_d[0, :, :])
            xt_sb = cpool.tile([128, DT * BS], BF16, tag="xt")
            for dd in range(DT):
                eng = nc.scalar if dd < 4 else nc.sync
                eng.dma_start(
                    xt_sb[:, dd * BS:(dd + 1) * BS], xt_d[:, dd * BS:(dd + 1) * BS]
                )
            b1t_sb = cpool.tile([128, T], F32, tag="b1t")
            nc.scalar.dma_start(b1t_sb[:], b1t_d[:])
            bg4_sb = cpool.tile([128, 1], F32, tag="bg4")
            nc.scalar.dma_start(bg4_sb[:], bg4_d[:])
            ob2_sb = cpool.tile([128, 2], BF16, tag="ob2")
            nc.scalar.dma_start(ob2_sb[:], ob2_d[:])
            w2c_sb = cpool.tile([128, T * 32], BF16, tag="w2c")
            nc.scalar.dma_start(w2c_sb[:], w2c_d[:])

            # gating is interleaved with stage-1 t=0 inside the main loop.
            # gps lives in the eo pool (bufs=1, so it aliases eo_ps): the
            # Exp read finishes by ~21us and eo's first burst (t=8, all
            # start=True) fully reinitializes the banks — and this keeps
            # ps1's two psh buffers free of WAR stalls against Exp.
            gps = pseo.tile([128, BS], F32, tag="eo")
            gate_order = [0, 4, 1, 5, 2, 6, 3, 7]
            u4 = cpool.tile([128, BS], BF16, tag="u4")
            u4f = cpool.tile([128, BS], F32, tag="u4f")

            # HAM warm-up: dependency-free matmuls on an uninitialized
            # scratch tile run during the initial DMA wait, so the PE is
            # at full clock (K=8/8) when real work arrives. Results are
            # overwritten by the gating group's start=True.
            scratch = cpool.tile([128, BH], BF16, tag="scratch")
            nc.gpsimd.memset(scratch[:], 0.0)
            for _ in range(16):
                nc.tensor.matmul(
                    gps[:, 0:BH],
                    scratch[:, 0:128],
                    scratch[:],
                    start=True, stop=True, skip_group_check=True,
                )

            # ---- main loop over t = ht*16 + e ----
            eo_ps = pseo.tile([128, BS], F32, tag="eo")
            pending = []  # [(t, hr), ...] up to BURST
            BURST = 8

            def emit_stage2_burst():
                # consecutive t's alternate col groups, so MMs overlap
                for bh in range(2):
                    for (tt, hh) in pending:
                        g = (tt % E) % 4
                        nc.tensor.matmul(
                            eo_ps[32 * g:32 * g + 32, bh * BH:(bh + 1) * BH],
                            w2c_sb[:, tt * 32:(tt + 1) * 32],
                            hh[:, bh * BH:(bh + 1) * BH],
                            start=(tt < 4), stop=(tt >= T - 4),
                            skip_group_check=True,
                            tile_position=(0, 32 * g),
                        )
                pending.clear()

            den_ps = pss.tile([128, NCH], F32, tag="sps")
            nb2_ps = pss.tile([128, NCH], F32, tag="sps2")
            rden = cpool.tile([128, NCH], F32, tag="rden")
            num2 = cpool.tile([128, NCH], F32, tag="num2")

            for t in range(T):
                if t == 0:
                    w1t = w1_first
                else:
                    w1t = w1pool.tile([128, DT * 128], BF16, tag="w1t")
                    nc.sync.dma_start(w1t[:], w1_d[t, :, :])
                ps1 = psh.tile([128, BS], F32, tag="ps1")
                # t=0: follow the two xt DMA streams, interleaving the
                # gating matmuls so the PE has work as tiles arrive
                dds = gate_order if t == 0 else range(DT)
                for i, dd in enumerate(dds):
                    if t == 0:
                        for bh in range(2):
                            nc.tensor.matmul(
                                gps[:, bh * BH:(bh + 1) * BH],
                                wgp_sb[:, dd * 128:(dd + 1) * 128],
                                xt_sb[:, dd * BS + bh * BH: dd * BS + (bh + 1) * BH],
                                start=(i == 0), stop=(i == DT - 1),
                                skip_group_check=True,
                            )
                    lhs = w1t[:, dd * 128:(dd + 1) * 128]
                    for bh in range(2):
                        nc.tensor.matmul(
                            ps1[:, bh * BH:(bh + 1) * BH],
                            lhs,
                            xt_sb[:, dd * BS + bh * BH: dd * BS + (bh + 1) * BH],
                            start=(i == 0), stop=(i == DT - 1),
                            skip_group_check=True,
                        )
                if t == 0:
                    nc.scalar.activation(u4[:], gps[:], AF.Exp, bias=bg4_sb[:])
                    nc.vector.tensor_copy(u4f[:], u4[:])
                if len(pending) == BURST or t == T - 1:
                    # at t == T-1 flush t'=120..126 now so only t=127's
                    # quarter-matmuls have to wait for the final ReLU
                    emit_stage2_burst()
                if t == 1:
                    # den/num2 partition-sums; u4 is ready by now, PE is warm
                    for j in range(NCH):
                        nc.tensor.matmul(
                            den_ps[:, j:j + 1],
                            u4[:, j * 128:(j + 1) * 128],
                            ob2_sb[:, 0:1],
                            start=True, stop=True, skip_group_check=True,
                        )
                        nc.tensor.matmul(
                            nb2_ps[:, j:j + 1],
                            u4[:, j * 128:(j + 1) * 128],
                            ob2_sb[:, 1:2],
                            start=True, stop=True, skip_group_check=True,
                        )
                if t == 2:
                    nc.vector.reciprocal(rden[:], den_ps[:])
                    nc.vector.tensor_copy(num2[:], nb2_ps[:])
                hr = hpool.tile([128, BS], BF16, tag="hr")
                # final tile: quarter-width ReLU chunks so the last stage-2
                # matmuls can chase them with minimal PE wait
                nq = 4 if t == T - 1 else 2
                w = BS // nq
                for q in range(nq):
                    nc.scalar.activation(
                        hr[:, q * w:(q + 1) * w],
                        ps1[:, q * w:(q + 1) * w],
                        AF.Relu,
                        bias=b1t_sb[:, t:t + 1],
                    )
                pending.append((t, hr))
            (tt, hh) = pending.pop()
            g = (tt % E) % 4
            for q in range(4):
                nc.tensor.matmul(
                    eo_ps[32 * g:32 * g + 32, q * 256:(q + 1) * 256],
                    w2c_sb[:, tt * 32:(tt + 1) * 32],
                    hh[:, q * 256:(q + 1) * 256],
                    start=False, stop=True,
                    skip_group_check=True,
                    tile_position=(0, 32 * g),
                )

            # ---- combine: V = U*eo -> num1 -> y = (num1+num2)*rden ----
            # V-mul split in halves so the partition-sum matmuls for the
            # first half overlap the DVE on the second half
            v4 = cpool.tile([128, BS], BF16, tag="v4")
            num_ps = pss.tile([128, NCH], F32, tag="sps")
            for qq in range(4):
                cols = slice(qq * 256, (qq + 1) * 256)
                nc.vector.tensor_mul(v4[:, cols], eo_ps[:, cols], u4f[:, cols])
                for j in (2 * qq, 2 * qq + 1):
                    nc.tensor.matmul(
                        num_ps[:, j:j + 1],
                        v4[:, j * 128:(j + 1) * 128],
                        ob2_sb[:, 0:1],
                        start=True, stop=True, skip_group_check=True,
                    )
            ysb = smpool.tile([128, NCH], F32, tag="ysb")
            nc.vector.tensor_add(ysb[:], num_ps[:], num2[:])
            nc.vector.tensor_mul(ysb[:], ysb[:], rden[:])
            nc.sync.dma_start(y_d[:], ysb[:])
    nc.compile()
    return nc


def prep_inputs(x, W1, b1, W2, b2, Wg, bg):
    """Host-side data prep. Returns (shared_map, per_core_xt)."""
    f = np.float32
    # W1 [E, D, H] -> [t=(ht,e), d_in, (d_t, h_in)]
    w1p = np.ascontiguousarray(
        W1.reshape(E, DT, 128, HT, 128).transpose(3, 0, 2, 1, 4)
        .reshape(T, 128, DT * 128)).astype(BF_NP)
    b1t = np.ascontiguousarray(
        b1.reshape(E, HT, 128).transpose(2, 1, 0).reshape(128, T).astype(f))
    # stage-2 stationaries: expert e -> col group g=e%4, row k=e//4
    w2c = np.zeros((128, T, 32), dtype=f)
    for t in range(T):
        ht, e = divmod(t, E)
        k, g = divmod(e, 4)
        w2c[:, t, k] = W2[e, ht * 128:(ht + 1) * 128]
    w2c = w2c.reshape(128, T * 32).astype(BF_NP)
    # gating stationary: col 32g+k = Wg[:, 4k+g], rest zero
    wgp4 = np.zeros((DT, 128, 128), dtype=f)
    bg4 = np.full((128, 1), -30.0, dtype=f)
    ob2 = np.zeros((128, 2), dtype=f)
    ob2[:, 0] = 1.0
    for e in range(E):
        k, g = divmod(e, 4)
        wgp4[:, :, 32 * g + k] = Wg[:, e].reshape(DT, 128)
        bg4[32 * g + k, 0] = bg[e]
        ob2[32 * g + k, 1] = b2[e]
    wgp4 = np.ascontiguousarray(
        wgp4.transpose(1, 0, 2).reshape(128, DT * 128)).astype(BF_NP)
    ob2 = ob2.astype(BF_NP)
    shared = {"w1p": w1p, "b1t": b1t, "w2c": w2c, "wgp4": wgp4,
              "bg4": bg4, "ob2": ob2}
    xT = np.ascontiguousarray(np.asarray(x, dtype=f).T)  # [D, B]
    xts = []
    for c in range(N_CORES):
        xc = xT[:, c * BS:(c + 1) * BS]  # [D, BS]
        xc = np.ascontiguousarray(
            xc.reshape(DT, 128, BS).transpose(1, 0, 2).reshape(128, DT * BS))
        xts.append(xc.astype(BF_NP))
    return shared, xts


def run(inputs, trace=False):
    nc = build_bass()
    shared, xts = prep_inputs(**inputs)
    in_maps = [dict(shared, xt=xts[c]) for c in range(N_CORES)]
    res = run_bass_kernel_spmd(
        nc, in_maps, core_ids=list(range(N_CORES)), trace=trace
    )
    # y dram is [128, NCH] with y[p, j] = out[j*128 + p]
    y = np.concatenate(
        [np.asarray(r["y"], dtype=np.float32).T.reshape(BS, 1)
         for r in res.results], axis=0)
    return y, res


def kernel(**inputs):
    y, _ = run(inputs, trace=False)
    return y


if __name__ == "__main__":
    rng = np.random.default_rng(0)
    ins = {
        "x": rng.standard_normal((B, D), dtype=np.float32),
        "W1": rng.standard_normal((E, D, H), dtype=np.float32) / 32,
        "b1": rng.standard_normal((E, H), dtype=np.float32) / 32,
        "W2": rng.standard_normal((E, H), dtype=np.float32) / 32,
        "b2": rng.standard_normal((E,), dtype=np.float32) / 32,
        "Wg": rng.standard_normal((D, E), dtype=np.float32) / 32,
        "bg": rng.standard_normal((E,), dtype=np.float32) / 32,
    }
    y = kernel(**ins)
    print("ok", y.shape, y.dtype)


# revision 39
# speedup vs baseline: 1.0031x; 1.0021x over previous
"""MoE kernel for TRN2, 8 NeuronCores, data-parallel over the batch dim.

Reference computation (B=8192, D=1024, H=1024, E=16):
    weights = softmax(x @ Wg + bg, axis=1)            # [B, E]
    h       = relu(einsum('bd,edh->beh', x, W1) + b1) # [B, E, H]
    eo      = einsum('beh,eh->be', h, W2) + b2        # [B, E]
    out     = sum(eo * weights, axis=1, keepdims=True)# [B, 1]

Strategy (618us baseline -> ~484us):
  - Shard B over 8 cores (1024 rows/core); weights replicated.
  - Everything in bf16 on the PE (tolerance 2e-2 vs ~4e-3 bf16 noise):
    halves W1 DMA traffic and enables fast weight load (FWL), so
    stage-1 matmuls issue at the ideal 216ns/N=512 spacing.
  - Stage 1 per t=(ht,e): psum[h=128, b=512x2] accumulated over 8
    d-tiles; W1 streamed from HBM on the sync HWDGE ring, 8 tiles deep.
  - ReLU+b1 on ScalarE -> hr bf16.
  - Stage 2 uses PE column tiling: experts are assigned to the 4 32-col
    groups (e -> group e%4, row e//4), so consecutive t's stage-2
    matmuls run CONCURRENTLY in disjoint col groups (~4x faster than a
    serialized block-diagonal form). Emitted in bursts of 16 t's, one t
    delayed so ReLU has time to land; the last tile is chased in
    quarter-width chunks to minimize the tail.
  - Gating: stationary Wg produces logits directly as [16e, B] (expert
    e at partition 32*(e%4)+e//4); U = exp(logits + bg) UNNORMALIZED.
    den = sum_e U and num2 = sum_e U*b2 via tiny partition-sum matmuls.
    Gating matmuls are interleaved with stage-1 t=0, following the two
    xt DMA streams (xt is split across both HWDGE rings).
  - 16 dependency-free warm-up matmuls on a scratch tile run during the
    initial DMA wait so the PE HAM clock-gate is at full rate (2.4GHz)
    when real work arrives.
  - Combine: V = U * eo (DVE, quarter-chased) -> num1 via 8
    partition-sum matmuls -> y = (num1 + num2) / den -> one [128,8] DMA.
"""

import ml_dtypes
import numpy as np

import concourse.bacc as bacc
import concourse.bass as bass
import concourse.mybir as mybir
from concourse import tile
from concourse.bass_utils import run_bass_kernel_spmd

B, D, H, E = 8192, 1024, 1024, 16
N_CORES = 8
BS = B // N_CORES  # 1024 batch rows per core
BH = 512           # half-batch moving-operand width (one psum bank)
DT = D // 128      # 8 d-tiles
HT = H // 128      # 8 h-tiles
T = E * HT         # 128 tiles; t = ht*16 + e  (e minor)
NCH = BS // 128    # 8 b-chunks of 128

F32 = mybir.dt.float32
BF16 = mybir.dt.bfloat16
AF = mybir.ActivationFunctionType
BF_NP = ml_dtypes.bfloat16


def build_bass():
    nc = bacc.Bacc("TRN2", target_bir_lowering=False, debug=False)
    xt_d = nc.dram_tensor("xt", [128, DT * BS], BF16, kind="ExternalInput")
    w1_d = nc.dram_tensor("w1p", [T, 128, DT * 128], BF16, kind="ExternalInput")
    b1t_d = nc.dram_tensor("b1t", [128, T], F32, kind="ExternalInput")
    w2c_d = nc.dram_tensor("w2c", [128, T * 32], BF16, kind="ExternalInput")
    wgp_d = nc.dram_tensor("wgp4", [128, DT * 128], BF16, kind="ExternalInput")
    bg4_d = nc.dram_tensor("bg4", [128, 1], F32, kind="ExternalInput")
    ob2_d = nc.dram_tensor("ob2", [128, 2], BF16, kind="ExternalInput")
    y_d = nc.dram_tensor("y", [128, NCH], F32, kind="ExternalOutput")

    with tile.TileContext(nc) as tc:
        with (
            tc.tile_pool(name="const", bufs=1) as cpool,
            tc.tile_pool(name="w1", bufs=8) as w1pool,
            tc.tile_pool(name="hrelu", bufs=12) as hpool,
            tc.tile_pool(name="sm", bufs=2) as smpool,
            tc.tile_pool(name="ps_h", bufs=2, space=bass.MemorySpace.PSUM) as psh,
            tc.tile_pool(name="ps_eo", bufs=1, space=bass.MemorySpace.PSUM) as pseo,
            tc.tile_pool(name="ps_s", bufs=1, space=bass.MemorySpace.PSUM) as pss,
        ):
            # ---- resident tensors; xt is split across BOTH HWDGE rings
            # (scalar=ACT gets d0..3, sync=SP gets d4..7 ahead of the W1
            # stream) so gating can start ~2x sooner ----
            # ring order: only wgp ahead of xt; all other consts after
            # (each dma_start costs ~0.7us of ring issue time)
            wgp_sb = cpool.tile([128, DT * 128], BF16, tag="wgp")
            nc.scalar.dma_start(wgp_sb[:], wgp_d[:])
            w1_first = w1pool.tile([128, DT * 128], BF16, tag="w1t")
            nc.sync.dma_start(w1_first[:], w1_d[0, :, :])
            # xt in 4 double-width transfers (pairs of d-tiles are
            # contiguous in dram): halves the per-DMA ring issue cost in
            # the congested startup window
            xt_sb = cpool.tile([128, DT * BS], BF16, tag="xt")
            for dp in range(4):
                eng = nc.scalar if dp < 2 else nc.sync
                eng.dma_start(
                    xt_sb[:, dp * 2 * BS:(dp + 1) * 2 * BS],
                    xt_d[:, dp * 2 * BS:(dp + 1) * 2 * BS],
                )
            b1t_sb = cpool.tile([128, T], F32, tag="b1t")
            nc.scalar.dma_start(b1t_sb[:], b1t_d[:])
            bg4_sb = cpool.tile([128, 1], F32, tag="bg4")
            nc.scalar.dma_start(bg4_sb[:], bg4_d[:])
            ob2_sb = cpool.tile([128, 2], BF16, tag="ob2")
            nc.scalar.dma_start(ob2_sb[:], ob2_d[:])
            w2c_sb = cpool.tile([128, T * 32], BF16, tag="w2c")
            nc.scalar.dma_start(w2c_sb[:], w2c_d[:])

            # gating is interleaved with stage-1 t=0 inside the main loop.
            # gps lives in the eo pool (bufs=1, so it aliases eo_ps): the
            # Exp read finishes by ~21us and eo's first burst (t=8, all
            # start=True) fully reinitializes the banks — and this keeps
            # ps1's two psh buffers free of WAR stalls against Exp.
            gps = pseo.tile([128, BS], F32, tag="eo")
            gate_order = [0, 1, 4, 5, 2, 3, 6, 7]
            u4 = cpool.tile([128, BS], BF16, tag="u4")
            u4f = cpool.tile([128, BS], F32, tag="u4f")

            # HAM warm-up: dependency-free matmuls on an uninitialized
            # scratch tile run during the initial DMA wait, so the PE is
            # at full clock (K=8/8) when real work arrives. Results are
            # overwritten by the gating group's start=True.
            scratch = cpool.tile([128, BH], BF16, tag="scratch")
            nc.gpsimd.memset(scratch[:], 0.0)
            for _ in range(16):
                nc.tensor.matmul(
                    gps[:, 0:BH],
                    scratch[:, 0:128],
                    scratch[:],
                    start=True, stop=True, skip_group_check=True,
                )

            # ---- main loop over t = ht*16 + e ----
            eo_ps = pseo.tile([128, BS], F32, tag="eo")
            pending = []  # [(t, hr), ...] up to BURST
            BURST = 8

            def emit_stage2_burst():
                # consecutive t's alternate col groups, so MMs overlap
                for bh in range(2):
                    for (tt, hh) in pending:
                        g = (tt % E) % 4
                        nc.tensor.matmul(
                            eo_ps[32 * g:32 * g + 32, bh * BH:(bh + 1) * BH],
                            w2c_sb[:, tt * 32:(tt + 1) * 32],
                            hh[:, bh * BH:(bh + 1) * BH],
                            start=(tt < 4), stop=(tt >= T - 4),
                            skip_group_check=True,
                            tile_position=(0, 32 * g),
                        )
                pending.clear()

            den_ps = pss.tile([128, NCH], F32, tag="sps")
            nb2_ps = pss.tile([128, NCH], F32, tag="sps2")
            rden = cpool.tile([128, NCH], F32, tag="rden")
            num2 = cpool.tile([128, NCH], F32, tag="num2")

            for t in range(T):
                if t == 0:
                    w1t = w1_first
                else:
                    w1t = w1pool.tile([128, DT * 128], BF16, tag="w1t")
                    nc.sync.dma_start(w1t[:], w1_d[t, :, :])
                ps1 = psh.tile([128, BS], F32, tag="ps1")
                # t=0: follow the two xt DMA streams, interleaving the
                # gating matmuls so the PE has work as tiles arrive
                dds = gate_order if t == 0 else range(DT)
                for i, dd in enumerate(dds):
                    if t == 0:
                        for bh in range(2):
                            nc.tensor.matmul(
                                gps[:, bh * BH:(bh + 1) * BH],
                                wgp_sb[:, dd * 128:(dd + 1) * 128],
                                xt_sb[:, dd * BS + bh * BH: dd * BS + (bh + 1) * BH],
                                start=(i == 0), stop=(i == DT - 1),
                                skip_group_check=True,
                            )
                    lhs = w1t[:, dd * 128:(dd + 1) * 128]
                    for bh in range(2):
                        nc.tensor.matmul(
                            ps1[:, bh * BH:(bh + 1) * BH],
                            lhs,
                            xt_sb[:, dd * BS + bh * BH: dd * BS + (bh + 1) * BH],
                            start=(i == 0), stop=(i == DT - 1),
                            skip_group_check=True,
                        )
                if t == 0:
                    nc.scalar.activation(u4[:], gps[:], AF.Exp, bias=bg4_sb[:])
                    nc.vector.tensor_copy(u4f[:], u4[:])
                if len(pending) == BURST or t == T - 1:
                    # at t == T-1 flush the backlog now so only t=127's
                    # quarter-matmuls have to wait for the final ReLU
                    emit_stage2_burst()
                if t == 1:
                    # den/num2 partition-sums; u4 is ready by now, PE is warm
                    for j in range(NCH):
                        nc.tensor.matmul(
                            den_ps[:, j:j + 1],
                            u4[:, j * 128:(j + 1) * 128],
                            ob2_sb[:, 0:1],
                            start=True, stop=True, skip_group_check=True,
                        )
                        nc.tensor.matmul(
                            nb2_ps[:, j:j + 1],
                            u4[:, j * 128:(j + 1) * 128],
                            ob2_sb[:, 1:2],
                            start=True, stop=True, skip_group_check=True,
                        )
                if t == 2:
                    nc.vector.reciprocal(rden[:], den_ps[:])
                    nc.vector.tensor_copy(num2[:], nb2_ps[:])
                hr = hpool.tile([128, BS], BF16, tag="hr")
                # final tile: quarter-width ReLU chunks so the last stage-2
                # matmuls can chase them with minimal PE wait
                nq = 4 if t == T - 1 else 2
                w = BS // nq
                for q in range(nq):
                    nc.scalar.activation(
                        hr[:, q * w:(q + 1) * w],
                        ps1[:, q * w:(q + 1) * w],
                        AF.Relu,
                        bias=b1t_sb[:, t:t + 1],
                    )
                pending.append((t, hr))
            (tt, hh) = pending.pop()
            g = (tt % E) % 4
            for q in range(4):
                nc.tensor.matmul(
                    eo_ps[32 * g:32 * g + 32, q * 256:(q + 1) * 256],
                    w2c_sb[:, tt * 32:(tt + 1) * 32],
                    hh[:, q * 256:(q + 1) * 256],
                    start=False, stop=True,
                    skip_group_check=True,
                    tile_position=(0, 32 * g),
                )

            # ---- combine: V = U*eo -> num1 -> y = (num1+num2)*rden ----
            # V-mul split in halves so the partition-sum matmuls for the
            # first half overlap the DVE on the second half
            v4 = cpool.tile([128, BS], BF16, tag="v4")
            num_ps = pss.tile([128, NCH], F32, tag="sps")
            for qq in range(4):
                cols = slice(qq * 256, (qq + 1) * 256)
                nc.vector.tensor_mul(v4[:, cols], eo_ps[:, cols], u4f[:, cols])
                for j in (2 * qq, 2 * qq + 1):
                    nc.tensor.matmul(
                        num_ps[:, j:j + 1],
                        v4[:, j * 128:(j + 1) * 128],
                        ob2_sb[:, 0:1],
                        start=True, stop=True, skip_group_check=True,
                    )
            ysb = smpool.tile([128, NCH], F32, tag="ysb")
            nc.vector.tensor_add(ysb[:], num_ps[:], num2[:])
            nc.vector.tensor_mul(ysb[:], ysb[:], rden[:])
            nc.sync.dma_start(y_d[:], ysb[:])
    nc.compile()
    return nc


def prep_inputs(x, W1, b1, W2, b2, Wg, bg):
    """Host-side data prep. Returns (shared_map, per_core_xt)."""
    f = np.float32
    # W1 [E, D, H] -> [t=(ht,e), d_in, (d_t, h_in)]
    w1p = np.ascontiguousarray(
        W1.reshape(E, DT, 128, HT, 128).transpose(3, 0, 2, 1, 4)
        .reshape(T, 128, DT * 128)).astype(BF_NP)
    b1t = np.ascontiguousarray(
        b1.reshape(E, HT, 128).transpose(2, 1, 0).reshape(128, T).astype(f))
    # stage-2 stationaries: expert e -> col group g=e%4, row k=e//4
    w2c = np.zeros((128, T, 32), dtype=f)
    for t in range(T):
        ht, e = divmod(t, E)
        k, g = divmod(e, 4)
        w2c[:, t, k] = W2[e, ht * 128:(ht + 1) * 128]
    w2c = w2c.reshape(128, T * 32).astype(BF_NP)
    # gating stationary: col 32g+k = Wg[:, 4k+g], rest zero
    wgp4 = np.zeros((DT, 128, 128), dtype=f)
    bg4 = np.full((128, 1), -30.0, dtype=f)
    ob2 = np.zeros((128, 2), dtype=f)
    ob2[:, 0] = 1.0
    for e in range(E):
        k, g = divmod(e, 4)
        wgp4[:, :, 32 * g + k] = Wg[:, e].reshape(DT, 128)
        bg4[32 * g + k, 0] = bg[e]
        ob2[32 * g + k, 1] = b2[e]
    wgp4 = np.ascontiguousarray(
        wgp4.transpose(1, 0, 2).reshape(128, DT * 128)).astype(BF_NP)
    ob2 = ob2.astype(BF_NP)
    shared = {"w1p": w1p, "b1t": b1t, "w2c": w2c, "wgp4": wgp4,
              "bg4": bg4, "ob2": ob2}
    xT = np.ascontiguousarray(np.asarray(x, dtype=f).T)  # [D, B]
    xts = []
    for c in range(N_CORES):
        xc = xT[:, c * BS:(c + 1) * BS]  # [D, BS]
        xc = np.ascontiguousarray(
            xc.reshape(DT, 128, BS).transpose(1, 0, 2).reshape(128, DT * BS))
        xts.append(xc.astype(BF_NP))
    return shared, xts


def run(inputs, trace=False):
    nc = build_bass()
    shared, xts = prep_inputs(**inputs)
    in_maps = [dict(shared, xt=xts[c]) for c in range(N_CORES)]
    res = run_bass_kernel_spmd(
        nc, in_maps, core_ids=list(range(N_CORES)), trace=trace
    )
    # y dram is [128, NCH] with y[p, j] = out[j*128 + p]
    y = np.concatenate(
        [np.asarray(r["y"], dtype=np.float32).T.reshape(BS, 1)
         for r in res.results], axis=0)
    return y, res


def kernel(**inputs):
    y, _ = run(inputs, trace=False)
    return y


if __name__ == "__main__":
    rng = np.random.default_rng(0)
    ins = {
        "x": rng.standard_normal((B, D), dtype=np.float32),
        "W1": rng.standard_normal((E, D, H), dtype=np.float32) / 32,
        "b1": rng.standard_normal((E, H), dtype=np.float32) / 32,
        "W2": rng.standard_normal((E, H), dtype=np.float32) / 32,
        "b2": rng.standard_normal((E,), dtype=np.float32) / 32,
        "Wg": rng.standard_normal((D, E), dtype=np.float32) / 32,
        "bg": rng.standard_normal((E,), dtype=np.float32) / 32,
    }
    y = kernel(**ins)
    print("ok", y.shape, y.dtype)
